# revision 35
# baseline (speedup 1.0000x reference)
"""GQA kernel for Trainium2, 8 NeuronCores.

Problem: x[1,4096,2048], H=16 heads, G=4 kv-groups, D=128, causal mask,
RoPE on q/k, out-proj. Sharding: 2 heads per core (core c -> heads 2c,2c+1,
kv-group c//2); out-proj sharded by output feature rows (core c -> dout
rows c*256..c*256+255, all 4096 positions).

Pipeline (all fp16 data path, fp32 PSUM accumulation):
  per q-chunk of 512:  project Q/K/V (K and V share one PSUM bank
  sequentially), RoPE on DVE, causal attention with scores one k-tile ahead
  of the PV matmuls (ACT exp latency hidden), softmax denominator via
  all-ones matmul (partition reduce+broadcast in one instruction) and
  1/d = exp(-ln d) on ACT. Context rows go out via a per-chunk AllGather
  that runs on the CC stream while compute continues; the out-projection
  for chunk qc-3 is interleaved into iteration qc, so phase 3 has no
  serial tail beyond the last chunk's gather.

Host supplies partition-major pre-layouts so every DMA moves >=8KB
contiguous per partition.
"""

import sys

for _p in ("/opt/trn_rl_repo",):
    if _p not in sys.path:
        sys.path.append(_p)

from contextlib import ExitStack

import numpy as np

import concourse.bass as bass
import concourse.tile as tile
from concourse import mybir
from concourse.bass_utils import run_bass_kernel_spmd

F32 = mybir.dt.float32
F16 = mybir.dt.float16
S = 4096
MAX_WAITS = 1  # walrus CoreV3 rejects instructions with more sync waits


def _split_sync_waits(nc, maxw=MAX_WAITS):
    """Hoist excess sem waits onto NOPs inserted before the instruction on
    the same engine queue (queue order makes this equivalent)."""
    from concourse import mybir as mb
    n = 0
    for bassbb in nc.bb_map.values():
        bb = bassbb.bb
        insts = list(bb.instructions)
        out = []
        changed = False
        for ins in insts:
            si = ins.sync_info
            if si is not None and si.on_wait and len(si.on_wait) > maxw:
                waits = list(si.on_wait)
                head, rest = waits[:-maxw], waits[-maxw:]
                while head:
                    chunk, head = head[:maxw], head[maxw:]
                    n += 1
                    nop = mb.InstNoOp(
                        name=f"I-ws{n}",
                        engine=ins.engine,
                        ins=[],
                        outs=[],
                        sync_info=mb.SyncInfo(on_wait=chunk, on_update=[]),
                    )
                    nc.register_instruction(nop)
                    out.append(nop)
                ins.sync_info = mb.SyncInfo(
                    on_wait=rest, on_update=list(si.on_update or []))
                changed = True
            out.append(ins)
        if changed:
            try:
                bb.instructions[:] = out
            except TypeError:
                bb.set_instructions(out)
    return n


DIN = 2048
D = 128
HPC = 2          # heads per core
DOUT_PC = HPC * D  # out-proj rows per core
NCORES = 8
QC = 512         # q-chunk (free dim per matmul)
NQ = S // QC     # 8 q-chunks
KT = 128         # k tile (partition dim)
NKIN = DIN // 128  # 16 contraction tiles for projections
NDT = DOUT_PC // 128  # 2 dout tiles per core
INV_SQRT_D = 1.0 / np.sqrt(D)
EXP_BIAS = -2.0  # keeps fp16 softmax sums well inside range; cancels in norm
OP_LAG = 3       # out-proj trails attention by this many chunks


def build_nc():
    nc = bass.Bass(num_devices=NCORES)

    # partition-major pre-layouts (big contiguous runs per partition)
    xP = nc.dram_tensor("xP", [128, NQ, NKIN, QC], F16, kind="ExternalInput")
    wqP = nc.dram_tensor("wqP", [128, NKIN, HPC * D], F16, kind="ExternalInput")
    wkP = nc.dram_tensor("wkP", [128, NKIN, D], F16, kind="ExternalInput")
    wvP = nc.dram_tensor("wvP", [128, NKIN, D], F16, kind="ExternalInput")
    woP = nc.dram_tensor("woP", [128, NKIN, DOUT_PC], F16, kind="ExternalInput")
    cosT = nc.dram_tensor("cosT", [D, S], F16, kind="ExternalInput")
    sinT = nc.dram_tensor("sinT", [D, S], F16, kind="ExternalInput")
    outP = nc.dram_tensor("outP", [128, NDT, S], F32, kind="ExternalOutput")

    # exchange buffers (collectives can't touch I/O tensors)
    cc_in = nc.dram_tensor("cc_in", [NQ, HPC * D, QC], F16)
    cc_all = nc.dram_tensor("cc_all", [NQ, NCORES, HPC * D, QC], F16,
                            addr_space="Shared")

    with ExitStack() as ctx:
        tc = ctx.enter_context(tile.TileContext(nc))

        res = ctx.enter_context(tc.tile_pool(name="res", bufs=1))
        # resident SBUF tensors
        qt = res.tile([128, HPC, S], F16, tag="qt")          # QT per head
        kt = res.tile([128, S], F16, tag="kt")               # KT (shared group)
        vt = res.tile([128, S // 128, D], F16, tag="vt")     # V as s-tiles
        wq_sb = res.tile([128, NKIN, HPC * D], F16, tag="wq")
        wk_sb = res.tile([128, NKIN, D], F16, tag="wk")
        wv_sb = res.tile([128, NKIN, D], F16, tag="wv")
        wo_sb = res.tile([128, NKIN, DOUT_PC], F16, tag="wo")
        cos_sb = res.tile([128, S], F16, tag="cos")
        sin_sb = res.tile([128, S], F16, tag="sin")
        ebias = res.tile([128, 1], F32, tag="ebias")         # exp bias const
        ones128 = res.tile([128, 128], F16, tag="ones128")   # partition reduce+bcast

        nc.vector.memset(ebias, EXP_BIAS)
        nc.vector.memset(ones128, 1.0)

        nc.sync.dma_start(out=wq_sb, in_=wqP[:, :, :])
        nc.sync.dma_start(out=wk_sb, in_=wkP[:, :, :])
        nc.sync.dma_start(out=wv_sb, in_=wvP[:, :, :])
        nc.sync.dma_start(out=wo_sb, in_=woP[:, :, :])
        nc.sync.dma_start(out=cos_sb, in_=cosT[:, :])
        nc.sync.dma_start(out=sin_sb, in_=sinT[:, :])

        # ---------------- fused main loop ----------------
        with ExitStack() as p2:
            xpool = p2.enter_context(tc.tile_pool(name="xpool", bufs=2))
            rpool = p2.enter_context(tc.tile_pool(name="rope", bufs=3))
            wpool = p2.enter_context(tc.tile_pool(name="wpool", bufs=6))
            apool = p2.enter_context(tc.tile_pool(name="acc", bufs=2))
            npool = p2.enter_context(tc.tile_pool(name="norm", bufs=2))
            copool = p2.enter_context(tc.tile_pool(name="cout", bufs=2))
            cpool = p2.enter_context(tc.tile_pool(name="cpool", bufs=2))
            opool = p2.enter_context(tc.tile_pool(name="opool", bufs=2))
            # PSUM: 8 banks of [128, 2KB]:
            #   pq/po shared tag (2) + pkv (1) + ps (2) + pd (1) + pc (2)
            pq_pool = p2.enter_context(tc.tile_pool(name="pq", bufs=2, space="PSUM"))
            pkv_pool = p2.enter_context(tc.tile_pool(name="pkv", bufs=1, space="PSUM"))
            ps_pool = p2.enter_context(tc.tile_pool(name="ps", bufs=2, space="PSUM"))
            pc_pool = p2.enter_context(tc.tile_pool(name="pc", bufs=2, space="PSUM"))

            def load_xt(qc):
                t = xpool.tile([128, NKIN, QC], F16, tag="xt")
                nc.sync.dma_start(out=t, in_=xP[:, qc, :, :])
                return t

            def emit_outproj(qc):
                """out-proj of chunk qc (gathered OP_LAG chunks ago)."""
                cc_sb = cpool.tile([128, NKIN, QC], F16, tag="cc_sb",
                                   name="cc_sb")
                nc.sync.dma_start(
                    out=cc_sb,
                    in_=cc_all[qc].rearrange("c (t p) m -> p (c t) m", p=128))
                for dt in range(NDT):
                    po = pq_pool.tile([128, QC], F32, tag="pq", name="po")
                    for ki in range(NKIN):
                        nc.tensor.matmul(
                            po, lhsT=wo_sb[:, ki, dt * 128:(dt + 1) * 128],
                            rhs=cc_sb[:, ki, :], start=(ki == 0),
                            stop=(ki == NKIN - 1))
                    ot = opool.tile([128, QC], F32, tag="ot", name="ot")
                    nc.scalar.copy(ot, po)
                    nc.sync.dma_start(
                        out=outP[:, dt, qc * QC:(qc + 1) * QC], in_=ot)

            xt_next = load_xt(0)
            for qc in range(NQ):
                q0 = qc * QC
                # -------- projections --------
                xt = xt_next
                q2 = rpool.tile([128, HPC, QC], F16, tag="q2")
                for h in range(HPC):
                    pq = pq_pool.tile([128, QC], F32, tag="pq")
                    for ki in range(NKIN):
                        nc.tensor.matmul(
                            pq, lhsT=wq_sb[:, ki, h * D:(h + 1) * D],
                            rhs=xt[:, ki, :], start=(ki == 0),
                            stop=(ki == NKIN - 1))
                    nc.vector.tensor_copy(q2[:, h, :], pq)
                # K then V through one PSUM bank (freed by the k1 copy)
                pk = pkv_pool.tile([128, QC], F32, tag="pkv", name="pk")
                for ki in range(NKIN):
                    nc.tensor.matmul(pk, lhsT=wk_sb[:, ki, :], rhs=xt[:, ki, :],
                                     start=(ki == 0), stop=(ki == NKIN - 1))
                k1 = rpool.tile([128, QC], F16, tag="k1")
                nc.vector.tensor_copy(k1, pk)
                pvt = pkv_pool.tile([128, QC], F32, tag="pkv", name="pvt")
                for ki in range(NKIN):
                    nc.tensor.matmul(pvt, lhsT=wv_sb[:, ki, :], rhs=xt[:, ki, :],
                                     start=(ki == 0), stop=(ki == NKIN - 1))
                # prefetch next x-chunk while this chunk's attention runs
                if qc + 1 < NQ:
                    xt_next = load_xt(qc + 1)

                # VT -> V via DMA XBAR transpose (fp16, runs on DMA engines:
                # out[p, m, d] = in[d, m*128 + p], exactly vt's layout)
                vtT = rpool.tile([128, QC], F16, tag="vtT")
                nc.vector.tensor_copy(vtT, pvt)
                nc.sync.dma_start_transpose(
                    out=vt[:, qc * 4:(qc + 1) * 4, :], in_=vtT)

                # -------- RoPE (all-fp16 on DVE) --------
                cos_c = cos_sb[:, q0:q0 + QC]
                sin_c = sin_sb[:, q0:q0 + QC]

                def rope(dst, src):
                    rot = rpool.tile([128, QC], F16, tag="rot")
                    nc.vector.tensor_scalar_mul(rot[0:64, :], src[64:128, :], -1.0)
                    nc.vector.tensor_copy(rot[64:128, :], src[0:64, :])
                    nc.vector.tensor_mul(dst, src, cos_c)
                    nc.vector.tensor_mul(rot, rot, sin_c)
                    nc.vector.tensor_add(dst, dst, rot)

                for h in range(HPC):
                    rope(qt[:, h, q0:q0 + QC], q2[:, h, :])
                rope(kt[:, q0:q0 + QC], k1)

                # -------- attention for this q-chunk --------
                nk = (qc + 1) * 4
                LAG = 1
                pcs = [pc_pool.tile([128, QC], F32, tag="pc", name=f"pc{h}")
                       for h in range(HPC)]
                accs = [apool.tile([128, QC], F16, tag="acc", name=f"acc{h}")
                        for h in range(HPC)]
                wts = {}

                def emit_scores(h, ki):
                    k0 = ki * KT
                    ps = ps_pool.tile([128, QC], F32, tag="ps")
                    nc.tensor.matmul(ps, lhsT=kt[:, k0:k0 + KT],
                                     rhs=qt[:, h, q0:q0 + QC],
                                     start=True, stop=True)
                    wt = wpool.tile([128, QC], F16, tag="wt")
                    nc.scalar.activation(wt, ps,
                                         mybir.ActivationFunctionType.Exp,
                                         scale=INV_SQRT_D, bias=ebias)
                    if k0 + KT - 1 > q0:
                        # keep where (q0+j) - (k0+p) >= 0
                        nc.gpsimd.affine_select(
                            out=wt, in_=wt, pattern=[[1, QC]],
                            compare_op=mybir.AluOpType.is_ge, fill=0.0,
                            base=q0 - k0, channel_multiplier=-1)
                    wts[(h, ki)] = wt

                def emit_pv(h, ki):
                    wt = wts.pop((h, ki))
                    nc.tensor.matmul(pcs[h], lhsT=vt[:, ki, :], rhs=wt,
                                     start=(ki == 0), stop=(ki == nk - 1))
                    if ki == 0:
                        nc.vector.tensor_copy(accs[h], wt)
                    else:
                        nc.vector.tensor_add(accs[h], accs[h], wt)

                for ki in range(nk):
                    for h in range(HPC):
                        emit_scores(h, ki)
                    if ki >= LAG:
                        for h in range(HPC):
                            emit_pv(h, ki - LAG)
                for ki in range(nk - LAG, nk):
                    for h in range(HPC):
                        emit_pv(h, ki)
                for h in range(HPC):
                    # denominator: all-ones matmul reduces over partitions AND
                    # broadcasts the sum to every partition in one instruction
                    pd = ps_pool.tile([128, QC], F32, tag="pd", bufs=1)
                    nc.tensor.matmul(pd, lhsT=ones128, rhs=accs[h],
                                     start=True, stop=True)
                    # 1/d as exp(-ln(d)) on ACT
                    lg = npool.tile([128, QC], F32, tag="lg")
                    nc.scalar.activation(lg, pd,
                                         mybir.ActivationFunctionType.Ln,
                                         scale=1.0)
                    rec = npool.tile([128, QC], F16, tag="rec")
                    nc.scalar.activation(rec, lg,
                                         mybir.ActivationFunctionType.Exp,
                                         scale=-1.0)
                    cout = copool.tile([128, QC], F16, tag="cout")
                    nc.vector.tensor_mul(cout, pcs[h], rec)
                    nc.gpsimd.dma_start(
                        out=cc_in[qc, h * D:(h + 1) * D, :], in_=cout)

                # gather this chunk's context rows from all cores (runs on
                # the CC stream while the next chunks compute)
                nc.gpsimd.collective_compute(
                    "AllGather",
                    mybir.AluOpType.bypass,
                    replica_groups=[list(range(NCORES))],
                    ins=[cc_in[qc]],
                    outs=[cc_all[qc]],
                )

                # out-proj of the chunk gathered OP_LAG iterations ago
                if qc >= OP_LAG:
                    emit_outproj(qc - OP_LAG)

            for qc in range(NQ - OP_LAG, NQ):
                emit_outproj(qc)

    _split_sync_waits(nc)
    return nc


_NC_CACHE = None


def _get_nc():
    global _NC_CACHE
    if _NC_CACHE is None:
        _NC_CACHE = build_nc()
    return _NC_CACHE


def _pmajor(a2d):
    """[T*128, M] -> [128, T, M] with row t*128+p landing at [p, t]."""
    t = a2d.shape[0] // 128
    return np.ascontiguousarray(
        a2d.reshape(t, 128, a2d.shape[1]).transpose(1, 0, 2))


def _make_in_maps(x, cos, sin, Wq, Wk, Wv, Wo):
    xT = x.reshape(S, DIN).T.astype(np.float16)          # [DIN, S]
    xPm = _pmajor(xT)                                    # [128, 16, 4096]
    xP = np.ascontiguousarray(
        xPm.reshape(128, NKIN, NQ, QC).transpose(0, 2, 1, 3))
    cosT = np.ascontiguousarray(cos.T.astype(np.float16))
    sinT = np.ascontiguousarray(sin.T.astype(np.float16))
    in_maps = []
    for c in range(NCORES):
        g = c // 2
        in_maps.append({
            "xP": xP,
            "wqP": _pmajor(Wq[c * 256:(c + 1) * 256, :].T.astype(np.float16)),
            "wkP": _pmajor(Wk[g * 128:(g + 1) * 128, :].T.astype(np.float16)),
            "wvP": _pmajor(Wv[g * 128:(g + 1) * 128, :].T.astype(np.float16)),
            "woP": _pmajor(np.ascontiguousarray(
                Wo[c * 256:(c + 1) * 256, :]).T.astype(np.float16)),
            "cosT": cosT,
            "sinT": sinT,
        })
    return in_maps


def run(x, cos, sin, Wq, Wk, Wv, Wo, trace=False, tmpdir=None):
    nc = _get_nc()
    in_maps = _make_in_maps(x, cos, sin, Wq, Wk, Wv, Wo)
    res = run_bass_kernel_spmd(nc, in_maps, list(range(NCORES)), trace=trace,
                               tmpdir=tmpdir)
    out = np.empty((1, S, DIN), dtype=np.float32)
    for c in range(NCORES):
        op = res.results[c]["outP"]                      # [128, NDT, S]
        for dt in range(NDT):
            out[0, :, c * 256 + dt * 128:c * 256 + (dt + 1) * 128] = op[:, dt, :].T
    return out, res


def kernel(x, mask, cos, sin, Wq, Wk, Wv, Wo):
    out, _ = run(np.asarray(x, dtype=np.float32), np.asarray(cos, np.float32),
                 np.asarray(sin, np.float32), np.asarray(Wq, np.float32),
                 np.asarray(Wk, np.float32), np.asarray(Wv, np.float32),
                 np.asarray(Wo, np.float32))
    return out


# revision 36
# speedup vs baseline: 1.7047x; 1.7047x over previous
"""GQA kernel for Trainium2, 8 NeuronCores.

Problem: x[1,4096,2048], H=16 heads, G=4 kv-groups, D=128, causal mask,
RoPE on q/k, out-proj. Sharding: 2 heads per core (core c -> heads 2c,2c+1,
kv-group c//2); out-proj sharded by output feature rows (core c -> dout
rows c*256..c*256+255, all 4096 positions).

Pipeline (all fp16 data path, fp32 PSUM accumulation):
  per q-chunk of 512:  project Q/K/V (K and V share one PSUM bank
  sequentially), RoPE on DVE, causal attention with scores one k-tile ahead
  of the PV matmuls (ACT exp latency hidden), softmax denominator via
  all-ones matmul (partition reduce+broadcast in one instruction) and
  1/d = exp(-ln d) on ACT. Context rows go out via a per-chunk AllGather
  that runs on the CC stream while compute continues; the out-projection
  for chunk qc-3 is interleaved into iteration qc, so phase 3 has no
  serial tail beyond the last chunk's gather.

Host supplies partition-major pre-layouts so every DMA moves >=8KB
contiguous per partition.
"""

import sys

for _p in ("/opt/trn_rl_repo",):
    if _p not in sys.path:
        sys.path.append(_p)

from contextlib import ExitStack

import numpy as np

import concourse.bass as bass
import concourse.tile as tile
from concourse import mybir
from concourse.bass_utils import run_bass_kernel_spmd

F32 = mybir.dt.float32
F16 = mybir.dt.float16
S = 4096
MAX_WAITS = 1  # walrus CoreV3 rejects instructions with more sync waits


def _split_sync_waits(nc, maxw=MAX_WAITS):
    """Hoist excess sem waits onto NOPs inserted before the instruction on
    the same engine queue (queue order makes this equivalent)."""
    from concourse import mybir as mb
    n = 0
    for bassbb in nc.bb_map.values():
        bb = bassbb.bb
        insts = list(bb.instructions)
        out = []
        changed = False
        for ins in insts:
            si = ins.sync_info
            if si is not None and si.on_wait and len(si.on_wait) > maxw:
                waits = list(si.on_wait)
                head, rest = waits[:-maxw], waits[-maxw:]
                while head:
                    chunk, head = head[:maxw], head[maxw:]
                    n += 1
                    nop = mb.InstNoOp(
                        name=f"I-ws{n}",
                        engine=ins.engine,
                        ins=[],
                        outs=[],
                        sync_info=mb.SyncInfo(on_wait=chunk, on_update=[]),
                    )
                    nc.register_instruction(nop)
                    out.append(nop)
                ins.sync_info = mb.SyncInfo(
                    on_wait=rest, on_update=list(si.on_update or []))
                changed = True
            out.append(ins)
        if changed:
            try:
                bb.instructions[:] = out
            except TypeError:
                bb.set_instructions(out)
    return n


DIN = 2048
D = 128
HPC = 2          # heads per core
DOUT_PC = HPC * D  # out-proj rows per core
NCORES = 8
QC = 512         # q-chunk (free dim per matmul)
NQ = S // QC     # 8 q-chunks
KT = 128         # k tile (partition dim)
NKIN = DIN // 128  # 16 contraction tiles for projections
NDT = DOUT_PC // 128  # 2 dout tiles per core
INV_SQRT_D = 1.0 / np.sqrt(D)
EXP_BIAS = -2.0  # keeps fp16 softmax sums well inside range; cancels in norm
OP_LAG = 3       # out-proj trails attention by this many chunks


def build_nc():
    nc = bass.Bass(num_devices=NCORES)

    # partition-major pre-layouts (big contiguous runs per partition)
    xP = nc.dram_tensor("xP", [128, NQ, NKIN, QC], F16, kind="ExternalInput")
    wqP = nc.dram_tensor("wqP", [128, NKIN, HPC * D], F16, kind="ExternalInput")
    wkP = nc.dram_tensor("wkP", [128, NKIN, D], F16, kind="ExternalInput")
    wvP = nc.dram_tensor("wvP", [128, NKIN, D], F16, kind="ExternalInput")
    woP = nc.dram_tensor("woP", [128, NKIN, DOUT_PC], F16, kind="ExternalInput")
    cosT = nc.dram_tensor("cosT", [D, S], F16, kind="ExternalInput")
    sinT = nc.dram_tensor("sinT", [D, S], F16, kind="ExternalInput")
    outP = nc.dram_tensor("outP", [128, NDT, S], F32, kind="ExternalOutput")

    # exchange buffers (collectives can't touch I/O tensors)
    cc_in = nc.dram_tensor("cc_in", [NQ, HPC * D, QC], F16)
    cc_all = nc.dram_tensor("cc_all", [NQ, NCORES, HPC * D, QC], F16)

    with ExitStack() as ctx:
        tc = ctx.enter_context(tile.TileContext(nc))

        res = ctx.enter_context(tc.tile_pool(name="res", bufs=1))
        # resident SBUF tensors
        qt = res.tile([128, HPC, S], F16, tag="qt")          # QT per head
        kt = res.tile([128, S], F16, tag="kt")               # KT (shared group)
        vt = res.tile([128, S // 128, D], F16, tag="vt")     # V as s-tiles
        wq_sb = res.tile([128, NKIN, HPC * D], F16, tag="wq")
        wk_sb = res.tile([128, NKIN, D], F16, tag="wk")
        wv_sb = res.tile([128, NKIN, D], F16, tag="wv")
        wo_sb = res.tile([128, NKIN, DOUT_PC], F16, tag="wo")
        cos_sb = res.tile([128, S], F16, tag="cos")
        sin_sb = res.tile([128, S], F16, tag="sin")
        ebias = res.tile([128, 1], F32, tag="ebias")         # exp bias const
        ones128 = res.tile([128, 128], F16, tag="ones128")   # partition reduce+bcast

        nc.vector.memset(ebias, EXP_BIAS)
        nc.vector.memset(ones128, 1.0)

        nc.sync.dma_start(out=wq_sb, in_=wqP[:, :, :])
        nc.sync.dma_start(out=wk_sb, in_=wkP[:, :, :])
        nc.sync.dma_start(out=wv_sb, in_=wvP[:, :, :])
        nc.sync.dma_start(out=wo_sb, in_=woP[:, :, :])
        nc.sync.dma_start(out=cos_sb, in_=cosT[:, :])
        nc.sync.dma_start(out=sin_sb, in_=sinT[:, :])

        # ---------------- fused main loop ----------------
        with ExitStack() as p2:
            xpool = p2.enter_context(tc.tile_pool(name="xpool", bufs=2))
            rpool = p2.enter_context(tc.tile_pool(name="rope", bufs=3))
            wpool = p2.enter_context(tc.tile_pool(name="wpool", bufs=6))
            apool = p2.enter_context(tc.tile_pool(name="acc", bufs=2))
            npool = p2.enter_context(tc.tile_pool(name="norm", bufs=2))
            copool = p2.enter_context(tc.tile_pool(name="cout", bufs=2))
            cpool = p2.enter_context(tc.tile_pool(name="cpool", bufs=2))
            opool = p2.enter_context(tc.tile_pool(name="opool", bufs=2))
            # PSUM: 8 banks of [128, 2KB]:
            #   pq/po shared tag (2) + pkv (1) + ps (2) + pd (1) + pc (2)
            pq_pool = p2.enter_context(tc.tile_pool(name="pq", bufs=2, space="PSUM"))
            pkv_pool = p2.enter_context(tc.tile_pool(name="pkv", bufs=1, space="PSUM"))
            ps_pool = p2.enter_context(tc.tile_pool(name="ps", bufs=2, space="PSUM"))
            pc_pool = p2.enter_context(tc.tile_pool(name="pc", bufs=2, space="PSUM"))

            def load_xt(qc):
                t = xpool.tile([128, NKIN, QC], F16, tag="xt")
                nc.sync.dma_start(out=t, in_=xP[:, qc, :, :])
                return t

            def emit_outproj(qc):
                """out-proj of chunk qc (gathered OP_LAG chunks ago)."""
                cc_sb = cpool.tile([128, NKIN, QC], F16, tag="cc_sb",
                                   name="cc_sb")
                nc.sync.dma_start(
                    out=cc_sb,
                    in_=cc_all[qc].rearrange("c (t p) m -> p (c t) m", p=128))
                for dt in range(NDT):
                    po = pq_pool.tile([128, QC], F32, tag="pq", name="po")
                    for ki in range(NKIN):
                        nc.tensor.matmul(
                            po, lhsT=wo_sb[:, ki, dt * 128:(dt + 1) * 128],
                            rhs=cc_sb[:, ki, :], start=(ki == 0),
                            stop=(ki == NKIN - 1))
                    ot = opool.tile([128, QC], F32, tag="ot", name="ot")
                    nc.scalar.copy(ot, po)
                    nc.sync.dma_start(
                        out=outP[:, dt, qc * QC:(qc + 1) * QC], in_=ot)

            xt_next = load_xt(0)
            for qc in range(NQ):
                q0 = qc * QC
                # -------- projections --------
                xt = xt_next
                q2 = rpool.tile([128, HPC, QC], F16, tag="q2")
                for h in range(HPC):
                    pq = pq_pool.tile([128, QC], F32, tag="pq")
                    for ki in range(NKIN):
                        nc.tensor.matmul(
                            pq, lhsT=wq_sb[:, ki, h * D:(h + 1) * D],
                            rhs=xt[:, ki, :], start=(ki == 0),
                            stop=(ki == NKIN - 1))
                    nc.vector.tensor_copy(q2[:, h, :], pq)
                # K then V through one PSUM bank (freed by the k1 copy)
                pk = pkv_pool.tile([128, QC], F32, tag="pkv", name="pk")
                for ki in range(NKIN):
                    nc.tensor.matmul(pk, lhsT=wk_sb[:, ki, :], rhs=xt[:, ki, :],
                                     start=(ki == 0), stop=(ki == NKIN - 1))
                k1 = rpool.tile([128, QC], F16, tag="k1")
                nc.vector.tensor_copy(k1, pk)
                pvt = pkv_pool.tile([128, QC], F32, tag="pkv", name="pvt")
                for ki in range(NKIN):
                    nc.tensor.matmul(pvt, lhsT=wv_sb[:, ki, :], rhs=xt[:, ki, :],
                                     start=(ki == 0), stop=(ki == NKIN - 1))
                # prefetch next x-chunk while this chunk's attention runs
                if qc + 1 < NQ:
                    xt_next = load_xt(qc + 1)

                # VT -> V via DMA XBAR transpose (fp16, runs on DMA engines:
                # out[p, m, d] = in[d, m*128 + p], exactly vt's layout)
                vtT = rpool.tile([128, QC], F16, tag="vtT")
                nc.vector.tensor_copy(vtT, pvt)
                nc.sync.dma_start_transpose(
                    out=vt[:, qc * 4:(qc + 1) * 4, :], in_=vtT)

                # -------- RoPE (all-fp16 on DVE) --------
                cos_c = cos_sb[:, q0:q0 + QC]
                sin_c = sin_sb[:, q0:q0 + QC]

                def rope(dst, src):
                    rot = rpool.tile([128, QC], F16, tag="rot")
                    nc.vector.tensor_scalar_mul(rot[0:64, :], src[64:128, :], -1.0)
                    nc.vector.tensor_copy(rot[64:128, :], src[0:64, :])
                    nc.vector.tensor_mul(dst, src, cos_c)
                    nc.vector.tensor_mul(rot, rot, sin_c)
                    nc.vector.tensor_add(dst, dst, rot)

                for h in range(HPC):
                    rope(qt[:, h, q0:q0 + QC], q2[:, h, :])
                rope(kt[:, q0:q0 + QC], k1)

                # -------- attention for this q-chunk --------
                nk = (qc + 1) * 4
                LAG = 1
                pcs = [pc_pool.tile([128, QC], F32, tag="pc", name=f"pc{h}")
                       for h in range(HPC)]
                accs = [apool.tile([128, QC], F16, tag="acc", name=f"acc{h}")
                        for h in range(HPC)]
                wts = {}

                def emit_scores(h, ki):
                    k0 = ki * KT
                    ps = ps_pool.tile([128, QC], F32, tag="ps")
                    nc.tensor.matmul(ps, lhsT=kt[:, k0:k0 + KT],
                                     rhs=qt[:, h, q0:q0 + QC],
                                     start=True, stop=True)
                    wt = wpool.tile([128, QC], F16, tag="wt")
                    nc.scalar.activation(wt, ps,
                                         mybir.ActivationFunctionType.Exp,
                                         scale=INV_SQRT_D, bias=ebias)
                    if k0 + KT - 1 > q0:
                        # keep where (q0+j) - (k0+p) >= 0
                        nc.gpsimd.affine_select(
                            out=wt, in_=wt, pattern=[[1, QC]],
                            compare_op=mybir.AluOpType.is_ge, fill=0.0,
                            base=q0 - k0, channel_multiplier=-1)
                    wts[(h, ki)] = wt

                def emit_pv(h, ki):
                    wt = wts.pop((h, ki))
                    nc.tensor.matmul(pcs[h], lhsT=vt[:, ki, :], rhs=wt,
                                     start=(ki == 0), stop=(ki == nk - 1))
                    if ki == 0:
                        nc.vector.tensor_copy(accs[h], wt)
                    else:
                        nc.vector.tensor_add(accs[h], accs[h], wt)

                for ki in range(nk):
                    for h in range(HPC):
                        emit_scores(h, ki)
                    if ki >= LAG:
                        for h in range(HPC):
                            emit_pv(h, ki - LAG)
                for ki in range(nk - LAG, nk):
                    for h in range(HPC):
                        emit_pv(h, ki)
                for h in range(HPC):
                    # denominator: all-ones matmul reduces over partitions AND
                    # broadcasts the sum to every partition in one instruction
                    pd = ps_pool.tile([128, QC], F32, tag="pd", bufs=1)
                    nc.tensor.matmul(pd, lhsT=ones128, rhs=accs[h],
                                     start=True, stop=True)
                    # 1/d as exp(-ln(d)) on ACT
                    lg = npool.tile([128, QC], F32, tag="lg")
                    nc.scalar.activation(lg, pd,
                                         mybir.ActivationFunctionType.Ln,
                                         scale=1.0)
                    rec = npool.tile([128, QC], F16, tag="rec")
                    nc.scalar.activation(rec, lg,
                                         mybir.ActivationFunctionType.Exp,
                                         scale=-1.0)
                    cout = copool.tile([128, QC], F16, tag="cout")
                    nc.vector.tensor_mul(cout, pcs[h], rec)
                    nc.gpsimd.dma_start(
                        out=cc_in[qc, h * D:(h + 1) * D, :], in_=cout)

                # gather this chunk's context rows from all cores (runs on
                # the CC stream while the next chunks compute)
                nc.gpsimd.collective_compute(
                    "AllGather",
                    mybir.AluOpType.bypass,
                    replica_groups=[list(range(NCORES))],
                    ins=[cc_in[qc]],
                    outs=[cc_all[qc]],
                )

                # out-proj of the chunk gathered OP_LAG iterations ago
                if qc >= OP_LAG:
                    emit_outproj(qc - OP_LAG)

            for qc in range(NQ - OP_LAG, NQ):
                emit_outproj(qc)

    _split_sync_waits(nc)
    return nc


_NC_CACHE = None


def _get_nc():
    global _NC_CACHE
    if _NC_CACHE is None:
        _NC_CACHE = build_nc()
    return _NC_CACHE


def _pmajor(a2d):
    """[T*128, M] -> [128, T, M] with row t*128+p landing at [p, t]."""
    t = a2d.shape[0] // 128
    return np.ascontiguousarray(
        a2d.reshape(t, 128, a2d.shape[1]).transpose(1, 0, 2))


def _make_in_maps(x, cos, sin, Wq, Wk, Wv, Wo):
    xT = x.reshape(S, DIN).T.astype(np.float16)          # [DIN, S]
    xPm = _pmajor(xT)                                    # [128, 16, 4096]
    xP = np.ascontiguousarray(
        xPm.reshape(128, NKIN, NQ, QC).transpose(0, 2, 1, 3))
    cosT = np.ascontiguousarray(cos.T.astype(np.float16))
    sinT = np.ascontiguousarray(sin.T.astype(np.float16))
    in_maps = []
    for c in range(NCORES):
        g = c // 2
        in_maps.append({
            "xP": xP,
            "wqP": _pmajor(Wq[c * 256:(c + 1) * 256, :].T.astype(np.float16)),
            "wkP": _pmajor(Wk[g * 128:(g + 1) * 128, :].T.astype(np.float16)),
            "wvP": _pmajor(Wv[g * 128:(g + 1) * 128, :].T.astype(np.float16)),
            "woP": _pmajor(np.ascontiguousarray(
                Wo[c * 256:(c + 1) * 256, :]).T.astype(np.float16)),
            "cosT": cosT,
            "sinT": sinT,
        })
    return in_maps


def run(x, cos, sin, Wq, Wk, Wv, Wo, trace=False, tmpdir=None):
    nc = _get_nc()
    in_maps = _make_in_maps(x, cos, sin, Wq, Wk, Wv, Wo)
    res = run_bass_kernel_spmd(nc, in_maps, list(range(NCORES)), trace=trace,
                               tmpdir=tmpdir)
    out = np.empty((1, S, DIN), dtype=np.float32)
    for c in range(NCORES):
        op = res.results[c]["outP"]                      # [128, NDT, S]
        for dt in range(NDT):
            out[0, :, c * 256 + dt * 128:c * 256 + (dt + 1) * 128] = op[:, dt, :].T
    return out, res


def kernel(x, mask, cos, sin, Wq, Wk, Wv, Wo):
    out, _ = run(np.asarray(x, dtype=np.float32), np.asarray(cos, np.float32),
                 np.asarray(sin, np.float32), np.asarray(Wq, np.float32),
                 np.asarray(Wk, np.float32), np.asarray(Wv, np.float32),
                 np.asarray(Wo, np.float32))
    return out
